# revision 6
# baseline (speedup 1.0000x reference)
"""Bass/Tile TRN2 kernel for nn_DecoderGroupedQueryHeadAttentionAlibi.

Sharding (8 cores): core = (b, g) with b = core//2 in [0,4) (batch),
g = core%2 (head parity). Slot i in [0,8) on group g computes global head
2*i + g; kv head of slot i is g + 2*(i%2). The host sums the two parity
partials of the row-sliced output projection and adds bproj.

Per-core device program (scoresT layout: [s_partitions, t_free]):
  - q/k/v projections from host-pretransposed xT/weight tiles (bf16),
    woven between attention iterations so the scalar engine never idles
  - scores as 64x64 PE-array quads (4 concurrent tile_position matmuls)
  - per (slot, s-tile): ACT exp with the alibi linear bias folded into the
    per-partition activation bias; fully-future tiles use bias=0 (the alibi
    bias is zero above the diagonal), others get a DVE fix on the future
    prefix plus Toeplitz multipliers on diag/past regions
  - attn@v accumulation in psum [65, 2048]; row 64 (ones column in v) is
    the softmax denominator
  - per head-pair: denominator reciprocal, normalize, and a two-pass output
    projection (pass A for early pairs hidden under later heads' attention).
"""

import math
import numpy as np

# ---- problem constants (hardcoded; kernel.py must be self-contained) ----
B, T, C = 4, 2048, 1024
N_HEAD, N_KV_HEAD, HEAD_DIM = 16, 4, 64
NH = 8            # head slots per core
ST = T // 128     # 16 s-tiles
NCH = T // 512    # 4 t-chunks
KCT = C // 128    # 8 contraction tiles of 128
CUT_MARGIN = 10.0  # exp(-10) ~ 4.5e-5: dropped mass is ~1e-4 of denom

_START = 2.0 ** (-2.0 ** (-(math.log2(N_HEAD) - 3.0)))  # 0.7071...

SLOT_ORDER = [4, 5, 0, 1, 6, 7, 3, 2]


def _head_of_slot(i: int, g: int) -> int:
    return 2 * i + g


def _a_of_head(h: int) -> float:
    return (_START ** (h + 1)) / math.sqrt(HEAD_DIM)


# Loop bounds must be identical on every core (SPMD): use the widest cutoff
# over g for each head slot (g=1 heads have smaller slopes -> wider bands).
_CUTOFF = [CUT_MARGIN / min(_a_of_head(_head_of_slot(i, 0)),
                            _a_of_head(_head_of_slot(i, 1)))
           for i in range(NH)]
_N_EFF = [[min(NCH, int((128 * j + _CUTOFF[i]) // 512) + 1)
           for j in range(ST)] for i in range(NH)]
_J_FIRST = [[min(j for j in range(ST) if _N_EFF[i][j] > tcn)
             for tcn in range(NCH)] for i in range(NH)]
# per-slot Toeplitz table widths (max index read is cutoff+512, cap 2048)
_WREP_W = [min(2048, int(math.ceil(_CUTOFF[i])) + 512) for i in range(NH)]
_WREP_OFF = [sum(_WREP_W[:i]) for i in range(NH)]

_NC_CACHE = {}


def _split_multiwait(nc, mybir, max_waits=1):
    """walrus in this env encodes at most one sync-wait per instruction;
    split extras onto same-engine NoOps emitted just before."""
    for f in nc.m.functions:
        for bb in f.blocks:
            new = []
            for ins in bb.instructions:
                si = ins.sync_info
                conds = list(si.on_wait) if si is not None else []
                if len(conds) > max_waits:
                    for cond in conds[:-max_waits]:
                        n = mybir.InstNoOp(
                            name=nc.get_next_instruction_name(), ins=[], outs=[])
                        n.engine = ins.engine
                        n.sync_info = mybir.SyncInfo(on_wait=[cond], on_update=[])
                        new.append(n)
                    si.on_wait = conds[-max_waits:]
                new.append(ins)
            bb.instructions = new


def _build_nc():
    if "nc" in _NC_CACHE:
        return _NC_CACHE["nc"]
    from contextlib import ExitStack
    import concourse.bass as bass
    import concourse.tile as tile
    from concourse import mybir

    f32 = mybir.dt.float32
    bf16 = mybir.dt.bfloat16
    AF = mybir.ActivationFunctionType
    MUL = mybir.AluOpType.mult
    ADD = mybir.AluOpType.add
    MIN = mybir.AluOpType.min

    nc = bass.Bass()

    xT_d = nc.dram_tensor("xT", [C, T], bf16, kind="ExternalInput")
    wq_d = nc.dram_tensor("wqT", [C, NH * 64], bf16, kind="ExternalInput")
    wk_d = nc.dram_tensor("wkT", [C, 128], bf16, kind="ExternalInput")
    wv_d = nc.dram_tensor("wvT", [C, 128], bf16, kind="ExternalInput")
    wp_d = nc.dram_tensor("wpT", [NH * 64, C], bf16, kind="ExternalInput")
    wrep_d = nc.dram_tensor("wrep", [NH, 128, 2048], bf16, kind="ExternalInput")
    u_d = nc.dram_tensor("usb", [128, NH], f32, kind="ExternalInput")
    bias_d = nc.dram_tensor("biassb", [128, NH], f32, kind="ExternalInput")
    out_d = nc.dram_tensor("out", [T, C], f32, kind="ExternalOutput")

    xT_r = xT_d.rearrange("(k p) t -> p k t", p=128)
    wq_r = wq_d.rearrange("(k p) e -> p k e", p=128)
    wrep_r = wrep_d.rearrange("h p w -> p h w")

    with ExitStack() as es:
        tc = es.enter_context(tile.TileContext(nc))
        const = es.enter_context(tc.tile_pool(name="const", bufs=1))
        work = es.enter_context(tc.tile_pool(name="work", bufs=2))
        ebufp = es.enter_context(tc.tile_pool(name="ebuf", bufs=3))
        dstgp = es.enter_context(tc.tile_pool(name="dstg", bufs=1))
        outp = es.enter_context(tc.tile_pool(name="outp", bufs=2))
        dramd = es.enter_context(tc.tile_pool(name="dramd", bufs=1, space="DRAM"))
        ps = es.enter_context(tc.tile_pool(name="ps", bufs=1, space="PSUM"))
        ph1 = es.enter_context(tc.tile_pool(name="ph1", bufs=1))

        # ---- persistent tiles ----
        wp = const.tile([128, 4, C], bf16)
        wrep = const.tile([128, sum(_WREP_W)], bf16)
        usb = const.tile([128, NH], f32)
        biassb = const.tile([128, NH], f32)
        kRep = const.tile([128, 2, T], bf16)     # kv on both halves
        v_sb = const.tile([128, ST, 130], bf16)  # [s, j, (v_kv0|1|v_kv1|1)]
        qRep = const.tile([128, NH, T], bf16)    # slot i on both halves
        outT = const.tile([128, 4, T], bf16)     # [(2 slots d), pair, t]
        dstack = const.tile([128, 128], bf16)    # [(slot,tt), t_in] denom
        osbA = const.tile([128, ST, 1024], bf16)  # pass-A outproj partial
        warm = const.tile([128, 1], f32)

        ddrow = dramd.tile([NH, T], bf16)
        rdram = dramd.tile([NH, T], bf16)
        rd3 = rdram.rearrange("i (a b) -> i a b", b=128)

        # transient projection inputs
        xT = ph1.tile([128, KCT, T], bf16)
        wk = ph1.tile([128, KCT, 128], bf16)
        wq = ph1.tile([128, KCT, NH * 64], bf16)
        wv = ph1.tile([128, KCT, 128], bf16)

        def wr(i):  # per-slot Toeplitz slice accessor
            return wrep[:, _WREP_OFF[i]:_WREP_OFF[i] + _WREP_W[i]]

        # ---- prologue DMAs (dependency-critical order) ----
        nc.sync.dma_start(out=wk, in_=wk_d.rearrange("(k p) e -> p k e", p=128))
        # force the exp table load early so it overlaps the prologue DMAs
        nc.vector.memset(warm, 0.0)
        nc.scalar.activation(warm, warm, AF.Exp, bias=0.0, scale=1.0)
        for kc in range(KCT):
            nc.sync.dma_start(out=xT[:, kc, :], in_=xT_r[:, kc, :])
        nc.gpsimd.dma_start(out=wq[:, :, 256:384], in_=wq_r[:, :, 256:384])
        nc.gpsimd.dma_start(out=usb, in_=u_d[:])
        nc.gpsimd.dma_start(out=biassb, in_=bias_d[:])
        nc.gpsimd.dma_start(out=wv, in_=wv_d.rearrange("(k p) e -> p k e", p=128))
        nc.gpsimd.dma_start(out=wq[:, :, 0:128], in_=wq_r[:, :, 0:128])
        for pos in range(NH):
            i = SLOT_ORDER[pos]
            eng = nc.gpsimd if pos % 2 == 0 else nc.sync
            eng.dma_start(out=wr(i), in_=wrep_r[:, i, 0:_WREP_W[i]])
        nc.gpsimd.dma_start(out=wq[:, :, 384:512], in_=wq_r[:, :, 384:512])
        nc.gpsimd.dma_start(out=wq[:, :, 128:256], in_=wq_r[:, :, 128:256])
        nc.gpsimd.dma_start(out=wp, in_=wp_d.rearrange("(k p) e -> p k e", p=128))
        nc.vector.memset(v_sb[:, :, 64], 1.0)
        nc.vector.memset(v_sb[:, :, 129], 1.0)

        # ---- projection work items ----
        def k_proj_chunk(sc):
            pk = ps.tile([128, 1024], f32, tag="S", bufs=2)
            for kc in range(KCT):
                nc.tensor.matmul(
                    pk[:, 0:512], lhsT=wk[:, kc, :],
                    rhs=xT[:, kc, 512 * sc:512 * (sc + 1)],
                    start=(kc == 0), stop=(kc == KCT - 1))
            sl = slice(512 * sc, 512 * (sc + 1))
            nc.vector.tensor_copy(kRep[0:64, 0, sl], pk[0:64, 0:512])
            nc.vector.tensor_copy(kRep[64:128, 1, sl], pk[64:128, 0:512])
            nc.sync.dma_start(out=kRep[64:128, 0, sl], in_=kRep[0:64, 0, sl])
            nc.sync.dma_start(out=kRep[0:64, 1, sl], in_=kRep[64:128, 1, sl])

        def q_proj_chunk(p, tcn):
            pq = ps.tile([128, 1024], f32, tag="S", bufs=2)
            for kc in range(KCT):
                nc.tensor.matmul(
                    pq[:, 0:512], lhsT=wq[:, kc, 128 * p:128 * (p + 1)],
                    rhs=xT[:, kc, 512 * tcn:512 * (tcn + 1)],
                    start=(kc == 0), stop=(kc == KCT - 1))
            sl = slice(512 * tcn, 512 * (tcn + 1))
            nc.vector.tensor_copy(qRep[0:64, 2 * p, sl], pq[0:64, 0:512])
            nc.vector.tensor_copy(qRep[64:128, 2 * p + 1, sl], pq[64:128, 0:512])
            nc.sync.dma_start(out=qRep[64:128, 2 * p, sl],
                              in_=qRep[0:64, 2 * p, sl])
            nc.sync.dma_start(out=qRep[0:64, 2 * p + 1, sl],
                              in_=qRep[64:128, 2 * p + 1, sl])

        def v_proj_tile(st):
            pv = ps.tile([128, 1024], f32, tag="S", bufs=2)
            for kc in range(KCT):
                nc.tensor.matmul(
                    pv[:, 0:128], lhsT=xT[:, kc, 128 * st:128 * (st + 1)],
                    rhs=wv[:, kc, :],
                    start=(kc == 0), stop=(kc == KCT - 1))
            nc.vector.tensor_copy(v_sb[:, st, 0:64], pv[:, 0:64])
            nc.vector.tensor_copy(v_sb[:, st, 65:129], pv[:, 64:128])

        def passA_tile(tt):
            pp = ps.tile([128, 1024], f32, tag="S", bufs=2)
            for ec in range(2):
                for n_, kt in enumerate([2, 0]):
                    nc.tensor.matmul(
                        pp[:, 512 * ec:512 * (ec + 1)],
                        lhsT=outT[:, kt, 128 * tt:128 * (tt + 1)],
                        rhs=wp[:, kt, 512 * ec:512 * (ec + 1)],
                        start=(n_ == 0), stop=(n_ == 1))
            nc.vector.tensor_copy(osbA[:, tt, :], pp)

        # work item lists per slot position; items are emitted at the START
        # of iteration j once (j+2)*n//ST exceeds their index (lead-2 JIT:
        # an item must precede the first attention emission that reads it)
        slot_items = [[] for _ in range(NH)]
        # pos 0 = slot 4: v tiles 1..15 and k chunks 1..3 forced here
        slot_items[0] = ([lambda st=st: v_proj_tile(st) for st in range(1, 4)]
                         + [lambda: k_proj_chunk(1)]
                         + [lambda st=st: v_proj_tile(st) for st in range(4, 7)]
                         + [lambda: k_proj_chunk(2)]
                         + [lambda st=st: v_proj_tile(st) for st in range(7, 10)]
                         + [lambda: k_proj_chunk(3)]
                         + [lambda st=st: v_proj_tile(st) for st in range(10, 16)])
        # pos 1 = slot 5: q pair 0 (needed by slots 0,1 at pos 2,3)
        slot_items[1] = [lambda t=t: q_proj_chunk(0, t) for t in range(NCH)]
        # pos 2,3 = slots 0,1: q pair 3 (for slots 6,7)
        slot_items[2] = [lambda t=t: q_proj_chunk(3, t) for t in range(2)]
        slot_items[3] = [lambda t=t: q_proj_chunk(3, t) for t in range(2, 4)]
        # pos 4,5 = slots 6,7: q pair 1 (for slots 3,2)
        slot_items[4] = [lambda t=t: q_proj_chunk(1, t) for t in range(2)]
        slot_items[5] = [lambda t=t: q_proj_chunk(1, t) for t in range(2, 4)]
        # pos 6,7 = slots 3,2: output projection pass A (pairs 2, 0)
        slot_items[6] = [lambda tt=tt: passA_tile(tt) for tt in range(8)]
        slot_items[7] = [lambda tt=tt: passA_tile(tt) for tt in range(8, 16)]

        def finish_pair(p):
            # dstack rows for slots 2p, 2p+1 are populated; recip + normalize
            dstf = dstgp.tile([32, 128], f32, tag="dstf")
            nc.vector.tensor_copy(dstf, dstack[32 * p:32 * p + 32, :])
            rstf = dstgp.tile([32, 128], f32, tag="rstf")
            nc.vector.reciprocal(rstf, dstf)
            rstb = dstgp.tile([32, 128], bf16, tag="rstb")
            nc.vector.tensor_copy(rstb, rstf)
            for hh in range(2):
                i = 2 * p + hh
                nc.sync.dma_start(out=rd3[i], in_=rstb[16 * hh:16 * hh + 16, :])
            rrep = work.tile([128, T], bf16, tag="rrep")
            for hh in range(2):
                i = 2 * p + hh
                src = rdram[i:i + 1, :]
                src = bass.AP(tensor=src.tensor, offset=src.offset,
                              ap=[[0, 64]] + list(src.ap)[1:])
                nc.sync.dma_start(out=rrep[64 * hh:64 * hh + 64, :], in_=src)
            nc.vector.tensor_tensor(outT[:, p, :], outT[:, p, :], rrep, MUL)

        # ---- prologue compute: k(0), q pair2 (slot 4 needs all 4), v(0) ----
        k_proj_chunk(0)
        for t in range(NCH):
            q_proj_chunk(2, t)
        v_proj_tile(0)

        # ---- attention slots with woven work items ----
        for pos in range(NH):
            i = SLOT_ORDER[pos]
            p, half = i // 2, i % 2
            items = list(slot_items[pos])
            n_items = len(items)
            emitted = 0

            pa = ps.tile([65, T], f32, tag="pa", bufs=1)
            dmin = work.tile([128, 128], bf16, tag="dmin")
            nc.vector.tensor_scalar(dmin, wr(i)[:, 0:128],
                                    usb[:, i:i + 1], None, MIN)
            for j in range(ST):
                # weave: emit work items scheduled for this j (lead-2, so an
                # item lands before the attention emission that depends on it)
                want = min(n_items, (j + 2) * n_items // ST)
                while emitted < want:
                    items[emitted]()
                    emitted += 1
                ne = _N_EFF[i][j]
                lo = 128 * j          # t < lo : future region (bias 0)
                hi = 128 * (j + 1)    # t >= hi: past region (Toeplitz)
                E = ebufp.tile([128, T], bf16, tag="E")
                for sh in range(2):
                    c0, c1 = 2 * sh, min(ne, 2 * sh + 2)
                    if c0 >= c1:
                        continue
                    base, top = 512 * c0, 512 * c1
                    S = ps.tile([128, 1024], f32, tag="S", bufs=2)
                    for tcn in range(c0, c1):
                        rh = 64 * (tcn - c0)
                        o = 512 * (tcn - c0)
                        for kh in range(2):
                            nc.tensor.matmul(
                                S[64 * kh:64 * kh + 64, o:o + 512],
                                lhsT=kRep[rh:rh + 64, half,
                                          128 * j + 64 * kh:
                                          128 * j + 64 * kh + 64],
                                rhs=qRep[rh:rh + 64, i,
                                         512 * tcn:512 * (tcn + 1)],
                                start=True, stop=True)
                    wv_ = top - base
                    if lo >= top:
                        # whole tile is future: alibi bias is 0 there
                        nc.scalar.activation(
                            E[:, base:top], S[:, 0:wv_], AF.Exp,
                            bias=0.0, scale=0.125)
                    else:
                        nc.scalar.activation(
                            E[:, base:top], S[:, 0:wv_], AF.Exp,
                            bias=biassb[:, i:i + 1], scale=0.125)
                        if lo > base:  # future prefix: cancel the bias
                            nc.vector.tensor_scalar(
                                E[:, base:lo], E[:, base:lo],
                                usb[:, i:i + 1], None, MUL)
                        if lo >= base:  # diagonal tile lives here
                            nc.vector.tensor_tensor(
                                E[:, lo:hi], E[:, lo:hi], dmin, MUL)
                        seg0 = max(hi, base)
                        if seg0 < top:  # past region: Toeplitz multiplier
                            nc.vector.tensor_tensor(
                                E[:, seg0:top], E[:, seg0:top],
                                wr(i)[:, 128 + seg0 - hi:128 + top - hi],
                                MUL)
                    for tcn in range(c0, c1):
                        nc.tensor.matmul(
                            pa[:, 512 * tcn:512 * (tcn + 1)],
                            lhsT=v_sb[:, j, 65 * half:65 * half + 65],
                            rhs=E[:, 512 * tcn:512 * (tcn + 1)],
                            start=(j == _J_FIRST[i][tcn]), stop=(j == ST - 1),
                            skip_group_check=True)
            while emitted < n_items:
                items[emitted]()
                emitted += 1

            # copy-out: rows 0:64 -> outT half; row 64 -> denom
            st65 = dstgp.tile([65, T], bf16, tag="st65")
            nc.vector.tensor_copy(st65, pa[0:65, :])
            nc.sync.dma_start(out=outT[64 * half:64 * half + 64, p, :],
                              in_=st65[0:64, :])
            nc.sync.dma_start(out=ddrow[i:i + 1, :], in_=st65[64:65, :])
            nc.sync.dma_start(
                out=dstack[16 * i:16 * (i + 1), :],
                in_=ddrow[i].rearrange("(a b) -> a b", b=128))

            if pos == 1:
                finish_pair(2)
            elif pos == 3:
                finish_pair(0)
            elif pos == 5:
                finish_pair(3)
            elif pos == 7:
                finish_pair(1)

        # ---- output projection pass B (pairs 3, 1) + final add + store ----
        for tt in range(ST):
            pp = ps.tile([128, 1024], f32, tag="S", bufs=2)
            for ec in range(2):
                for n_, kt in enumerate([3, 1]):
                    nc.tensor.matmul(
                        pp[:, 512 * ec:512 * (ec + 1)],
                        lhsT=outT[:, kt, 128 * tt:128 * (tt + 1)],
                        rhs=wp[:, kt, 512 * ec:512 * (ec + 1)],
                        start=(n_ == 0), stop=(n_ == 1))
            osb = outp.tile([128, C], f32, tag="osb")
            nc.vector.tensor_tensor(osb, pp, osbA[:, tt, :], ADD)
            eng = nc.sync if tt % 2 == 0 else nc.gpsimd
            eng.dma_start(out=out_d[128 * tt:128 * (tt + 1), :], in_=osb)

    _split_multiwait(nc, mybir)
    _NC_CACHE["nc"] = nc
    return nc


def _prep_core_inputs(x, Wq, Wkv, Wproj, b, g):
    import ml_dtypes
    bf = ml_dtypes.bfloat16
    heads = [_head_of_slot(i, g) for i in range(NH)]
    xT = np.ascontiguousarray(x[b].T).astype(bf)                      # [C, T]
    wq_cols = np.concatenate([Wq[64 * h:64 * (h + 1)] for h in heads], axis=0)
    wqT = np.ascontiguousarray(wq_cols.T).astype(bf)                  # [C, 512]
    kv_rows = np.concatenate([np.arange(64 * kv, 64 * (kv + 1))
                              for kv in (g, g + 2)])
    wkT = np.ascontiguousarray(Wkv[kv_rows].T).astype(bf)             # [C, 128]
    wvT = np.ascontiguousarray(Wkv[256 + kv_rows].T).astype(bf)
    cols = np.concatenate([np.arange(64 * h, 64 * (h + 1)) for h in heads])
    wpT = np.ascontiguousarray(Wproj[:, cols].T).astype(bf)           # [512, C]

    s_in = np.arange(128, dtype=np.float64)
    wrep = np.zeros((NH, 128, 2048), dtype=bf)
    u = np.empty((128, NH), dtype=np.float32)
    bias = np.empty((128, NH), dtype=np.float32)
    idx = np.arange(2048, dtype=np.float64)
    for i, h in enumerate(heads):
        a = _a_of_head(h)
        w = _WREP_W[i]
        wrep[i, :, :w] = np.exp(-a * (idx[:w] - 127.0))[None, :].astype(np.float32)
        u[:, i] = np.exp(a * (127.0 - s_in)).astype(np.float32)
        bias[:, i] = (a * (s_in - 127.0)).astype(np.float32)
    return {"xT": xT, "wqT": wqT, "wkT": wkT, "wvT": wvT, "wpT": wpT,
            "wrep": wrep, "usb": u, "biassb": bias}


def kernel(x, Wq, Wkv, Wproj, bproj):
    from concourse.bass_utils import run_bass_kernel_spmd
    x = np.asarray(x, dtype=np.float32)
    Wq = np.asarray(Wq, dtype=np.float32)
    Wkv = np.asarray(Wkv, dtype=np.float32)
    Wproj = np.asarray(Wproj, dtype=np.float32)
    bproj = np.asarray(bproj, dtype=np.float32)

    nc = _build_nc()
    in_maps = [_prep_core_inputs(x, Wq, Wkv, Wproj, c // 2, c % 2)
               for c in range(8)]
    res = run_bass_kernel_spmd(nc, in_maps, core_ids=list(range(8)))
    out = np.zeros((B, T, C), dtype=np.float32)
    for c in range(8):
        out[c // 2] += res.results[c]["out"]
    out += bproj[None, None, :]
    return out
